# revision 5
# baseline (speedup 1.0000x reference)
"""Trainium2 Bass kernel for CustomBCELoss.

Reference semantics (per torch BCELoss with per-channel weighting):
    p, t flattened channel-first to (C=3, M=8388608)
    ones[c]   = count_nonzero(t[c])
    weight[c] = M / max(ones[c], 1)  if ones[c] > 0 else 1000.0
    bce[c]    = -mean(t*max(log p, -100) + (1-t)*max(log1p(-p), -100))
    out       = mean(weight * bce)

Since t ∈ {0,1}, the per-element term is log|p + t - 1|, and with
p ∈ [1e-4, 1-1e-4] (post-sigmoid probabilities) the -100 clamp never
fires: |p + t - 1| >= ~6e-5 so log >= ~-10.

Device pipeline per [128, 4096] tile (8-way data-parallel over the flat
element range; tiles never cross an (n, c) block boundary so per-tile
partial sums map 1:1 to channels on the host):
    DVE : tsum_col = reduce_sum(t)                 (positive count)
    DVE : d = (p - 1) + t                          (fused scalar_tensor_tensor)
    ACT : u = Abs(d)
    ACT : v = Ln(u), accum_out -> vsum_col         (fused per-partition sum)
Host combines the tiny [128, ntiles] partials per channel and applies the
weight/mean epilogue in float64.
"""

import numpy as np

import concourse.bacc as bacc
import concourse.bass as bass
import concourse.tile as tile
from concourse import mybir
from concourse.bass_utils import run_bass_kernel_spmd

N_CORES = 8
C = 3
SPATIAL = 128 * 128 * 128            # elements per (n, c) block
N_BATCH = 4
FULL = N_BATCH * C * SPATIAL         # 25_165_824 total elements
PER_CORE = FULL // N_CORES           # 3_145_728
P = 128
F = 4096
TILE_ELEMS = P * F                   # 524_288
NTILES = PER_CORE // TILE_ELEMS      # 6
M_PER_CH = FULL // C                 # 8_388_608
EMPTY_WEIGHT = 1000.0

_NC_CACHE = None


def _build_nc():
    nc = bacc.Bacc(
        "TRN2", target_bir_lowering=False, debug=False, num_devices=N_CORES
    )
    p_in = nc.declare_dram_parameter(
        "p_in", [NTILES, P, F], mybir.dt.float32, isOutput=False
    )
    t_in = nc.declare_dram_parameter(
        "t_in", [NTILES, P, F], mybir.dt.float32, isOutput=False
    )
    vsum_out = nc.declare_dram_parameter(
        "vsum", [P, NTILES], mybir.dt.float32, isOutput=True
    )
    tsum_out = nc.declare_dram_parameter(
        "tsum", [P, NTILES], mybir.dt.float32, isOutput=True
    )
    with tile.TileContext(nc) as tc:
        with (
            tc.tile_pool(name="io", bufs=3) as io_pool,
            tc.tile_pool(name="act", bufs=2) as act_pool,
            tc.tile_pool(name="res", bufs=1) as res_pool,
        ):
            vsum_t = res_pool.tile([P, NTILES], mybir.dt.float32)
            tsum_t = res_pool.tile([P, NTILES], mybir.dt.float32)
            for i in range(NTILES):
                p_t = io_pool.tile([P, F], mybir.dt.float32, tag="p")
                t_t = io_pool.tile([P, F], mybir.dt.float32, tag="t")
                u_t = act_pool.tile([P, F], mybir.dt.float32, tag="u")
                nc.sync.dma_start(out=p_t, in_=p_in[i])
                nc.sync.dma_start(out=t_t, in_=t_in[i])
                nc.vector.reduce_sum(
                    out=tsum_t[:, i : i + 1], in_=t_t, axis=mybir.AxisListType.X
                )
                # d = (p - 1) + t, in place into p_t
                nc.vector.scalar_tensor_tensor(
                    out=p_t,
                    in0=p_t,
                    scalar=1.0,
                    in1=t_t,
                    op0=mybir.AluOpType.subtract,
                    op1=mybir.AluOpType.add,
                )
                nc.scalar.activation(
                    out=u_t, in_=p_t, func=mybir.ActivationFunctionType.Abs
                )
                nc.scalar.activation(
                    out=u_t,
                    in_=u_t,
                    func=mybir.ActivationFunctionType.Ln,
                    accum_out=vsum_t[:, i : i + 1],
                )
            nc.sync.dma_start(out=vsum_out[:], in_=vsum_t)
            nc.sync.dma_start(out=tsum_out[:], in_=tsum_t)
    nc.compile()
    return nc


def _get_nc():
    global _NC_CACHE
    if _NC_CACHE is None:
        _NC_CACHE = _build_nc()
    return _NC_CACHE


def _run_device(input, target, **spmd_kwargs):
    p_flat = np.ascontiguousarray(input, dtype=np.float32).reshape(-1)
    t_flat = np.ascontiguousarray(target, dtype=np.float32).reshape(-1)
    in_maps = []
    for k in range(N_CORES):
        sl = slice(k * PER_CORE, (k + 1) * PER_CORE)
        in_maps.append(
            {
                "p_in": p_flat[sl].reshape(NTILES, P, F),
                "t_in": t_flat[sl].reshape(NTILES, P, F),
            }
        )
    return run_bass_kernel_spmd(nc=_get_nc(), in_maps=in_maps,
                                core_ids=list(range(N_CORES)), **spmd_kwargs)


def _epilogue(results):
    sum_v = np.zeros(C, dtype=np.float64)
    sum_t = np.zeros(C, dtype=np.float64)
    for k in range(N_CORES):
        vs = results[k]["vsum"].astype(np.float64)  # [P, NTILES]
        ts = results[k]["tsum"].astype(np.float64)
        for i in range(NTILES):
            g = k * PER_CORE + i * TILE_ELEMS
            ch = (g // SPATIAL) % C
            sum_v[ch] += vs[:, i].sum()
            sum_t[ch] += ts[:, i].sum()
    total = float(M_PER_CH)
    ones = sum_t
    weight = np.where(ones > 0, total / np.maximum(ones, 1.0), EMPTY_WEIGHT)
    bce = -sum_v / total
    return np.asarray((weight * bce).mean(), dtype=np.float32)


def kernel(input, target):
    res = _run_device(input, target)
    return _epilogue(res.results)


# revision 7
# speedup vs baseline: 1.0058x; 1.0058x over previous
"""Trainium2 Bass kernel for CustomBCELoss.

Reference semantics (per torch BCELoss with per-channel weighting):
    p, t flattened channel-first to (C=3, M=8388608)
    ones[c]   = count_nonzero(t[c])
    weight[c] = M / max(ones[c], 1)  if ones[c] > 0 else 1000.0
    bce[c]    = -mean(t*max(log p, -100) + (1-t)*max(log1p(-p), -100))
    out       = mean(weight * bce)

Since t ∈ {0,1}, the per-element term is log|p + t - 1|, and with
p ∈ [1e-4, 1-1e-4] (post-sigmoid probabilities) the -100 clamp never
fires: |p + t - 1| >= ~6e-5 so log >= ~-10.

Device pipeline per [128, F] tile (8-way data-parallel over the flat
element range; tiles never cross an (n, c) block boundary so per-tile
partial sums map 1:1 to channels on the host):
    DVE : t = t * 1, accum_out -> tsum_col      (2x-mode copy w/ fused sum)
    DVE : d = (p - 1) + t                       (fused scalar_tensor_tensor)
    ACT : u = Abs(d)
    ACT : v = Ln(u), accum_out -> vsum_col      (fused per-partition sum)
The final tiles are tapered (2048 cols) to shorten the pipeline-drain
tail. Host combines the tiny [128, ntiles] partials per channel and
applies the weight/mean epilogue in float64.
"""

import numpy as np

import concourse.bacc as bacc
import concourse.bass as bass
import concourse.tile as tile
from concourse import mybir
from concourse.bass_utils import run_bass_kernel_spmd

N_CORES = 8
C = 3
SPATIAL = 128 * 128 * 128            # elements per (n, c) block
N_BATCH = 4
FULL = N_BATCH * C * SPATIAL         # 25_165_824 total elements
PER_CORE = FULL // N_CORES           # 3_145_728
P = 128
# Per-partition column counts per tile; sum must equal PER_CORE / P = 24576.
TILE_F = [4096, 4096, 4096, 4096, 4096, 2048, 2048]
NTILES = len(TILE_F)
TILE_ELEMS = [P * f for f in TILE_F]
assert sum(TILE_ELEMS) == PER_CORE
M_PER_CH = FULL // C                 # 8_388_608
EMPTY_WEIGHT = 1000.0

_NC_CACHE = None


def _build_nc():
    nc = bacc.Bacc(
        "TRN2", target_bir_lowering=False, debug=False, num_devices=N_CORES
    )
    p_in = nc.declare_dram_parameter(
        "p_in", [PER_CORE], mybir.dt.float32, isOutput=False
    )
    t_in = nc.declare_dram_parameter(
        "t_in", [PER_CORE], mybir.dt.float32, isOutput=False
    )
    vsum_out = nc.declare_dram_parameter(
        "vsum", [P, NTILES], mybir.dt.float32, isOutput=True
    )
    tsum_out = nc.declare_dram_parameter(
        "tsum", [P, NTILES], mybir.dt.float32, isOutput=True
    )
    with tile.TileContext(nc) as tc:
        with (
            tc.tile_pool(name="io", bufs=3) as io_pool,
            tc.tile_pool(name="act", bufs=2) as act_pool,
            tc.tile_pool(name="res", bufs=1) as res_pool,
        ):
            vsum_t = res_pool.tile([P, NTILES], mybir.dt.float32)
            tsum_t = res_pool.tile([P, NTILES], mybir.dt.float32)
            off = 0
            for i, f in enumerate(TILE_F):
                n = P * f
                p_src = p_in[off : off + n].rearrange("(p f) -> p f", p=P)
                t_src = t_in[off : off + n].rearrange("(p f) -> p f", p=P)
                off += n
                p_t = io_pool.tile([P, f], mybir.dt.float32, tag="p")
                t_t = io_pool.tile([P, f], mybir.dt.float32, tag="t")
                u_t = act_pool.tile([P, f], mybir.dt.float32, tag="u")
                nc.sync.dma_start(out=p_t, in_=p_src)
                nc.sync.dma_start(out=t_t, in_=t_src)
                # in-place identity with fused sum: tsum_col = sum(t)
                nc.vector.tensor_scalar(
                    out=t_t,
                    in0=t_t,
                    scalar1=1.0,
                    scalar2=0.0,
                    op0=mybir.AluOpType.mult,
                    op1=mybir.AluOpType.add,
                    accum_out=tsum_t[:, i : i + 1],
                )
                # d = (p - 1) + t, in place into p_t
                nc.vector.scalar_tensor_tensor(
                    out=p_t,
                    in0=p_t,
                    scalar=1.0,
                    in1=t_t,
                    op0=mybir.AluOpType.subtract,
                    op1=mybir.AluOpType.add,
                )
                nc.scalar.activation(
                    out=u_t, in_=p_t, func=mybir.ActivationFunctionType.Abs
                )
                nc.scalar.activation(
                    out=u_t,
                    in_=u_t,
                    func=mybir.ActivationFunctionType.Ln,
                    accum_out=vsum_t[:, i : i + 1],
                )
            nc.sync.dma_start(out=vsum_out[:], in_=vsum_t)
            nc.sync.dma_start(out=tsum_out[:], in_=tsum_t)
    nc.compile()
    return nc


def _get_nc():
    global _NC_CACHE
    if _NC_CACHE is None:
        _NC_CACHE = _build_nc()
    return _NC_CACHE


def _run_device(input, target, **spmd_kwargs):
    p_flat = np.ascontiguousarray(input, dtype=np.float32).reshape(-1)
    t_flat = np.ascontiguousarray(target, dtype=np.float32).reshape(-1)
    in_maps = []
    for k in range(N_CORES):
        sl = slice(k * PER_CORE, (k + 1) * PER_CORE)
        in_maps.append({"p_in": p_flat[sl], "t_in": t_flat[sl]})
    return run_bass_kernel_spmd(nc=_get_nc(), in_maps=in_maps,
                                core_ids=list(range(N_CORES)), **spmd_kwargs)


def _epilogue(results):
    sum_v = np.zeros(C, dtype=np.float64)
    sum_t = np.zeros(C, dtype=np.float64)
    for k in range(N_CORES):
        vs = results[k]["vsum"].astype(np.float64)  # [P, NTILES]
        ts = results[k]["tsum"].astype(np.float64)
        off = 0
        for i, n in enumerate(TILE_ELEMS):
            g = k * PER_CORE + off
            off += n
            ch = (g // SPATIAL) % C
            sum_v[ch] += vs[:, i].sum()
            sum_t[ch] += ts[:, i].sum()
    total = float(M_PER_CH)
    ones = sum_t
    weight = np.where(ones > 0, total / np.maximum(ones, 1.0), EMPTY_WEIGHT)
    bce = -sum_v / total
    return np.asarray((weight * bce).mean(), dtype=np.float32)


def kernel(input, target):
    res = _run_device(input, target)
    return _epilogue(res.results)
